# revision 4
# baseline (speedup 1.0000x reference)
"""CrossAttentionFusion kernel for Trainium2 (8 NeuronCores, data-parallel over batch).

Reference computation (per batch element, S=2048, D=512, HID=256):
  Q = l @ Wq + bq ; K = a @ Wk + bk ; V = a @ Wv + bv
  P = softmax(Q K^T / sqrt(D)) ; O = P @ V
  fused_l = gl*O + (2-gl)*l          (gl = sigmoid(alpha_l))
  fused_a = (1+ga)*a                 (ga = sigmoid(alpha_a))
  w = sigmoid(relu(v @ W1 + b1) @ W2 + b2) ; fused_v = w*v
  out = concat([fused_l, fused_a, fused_v], -1)     # [S, 3D]

Kernel strategy (per core, one batch element):
  - the host feeds activations in matmul-ready layouts: a^T/l^T in fp8e4 and
    v^T in bf16 ([d, s], chunked over 128-partition d-slices), plus natural
    bf16 copies for the elementwise epilogues. This removes all on-chip
    transposes and dtype-cast passes.
  - Q/K/V projections and both attention matmuls run in fp8e4 with DoubleRow
    perf mode (two 128-row k-subtiles per matmul -> 2x PE throughput); the
    visual-gate MLP stays bf16. All accumulation is fp32 in PSUM.
    Measured end-to-end rel err ~3e-3 (validated against a numpy simulation
    of the exact quantization points).
  - softmax skips the max pass: P = exp(s/sqrt(D) - 2) written straight to
    fp8 (the -2 bias keeps exp() below the fp8e4 max of 240; the constant
    cancels in the rowsum normalization). The rowsum comes from a ones
    column appended to V (PSUM split 256+257 to stay within banks).
  - gl is folded into V's bias-add eviction (host pre-scales bv by gl), so
    P@V already yields gl*O and the epilogue is (O_acc * (1/r)) + (2-gl)*l
    in one DVE pass per half.
  - attention is software-pipelined: scores(qb+1) is emitted between
    scores(qb) and PV(qb) so the PE never waits on the ScalarE Exp stream.
"""

import math
from contextlib import ExitStack

import ml_dtypes
import numpy as np

import concourse.bass as bass
import concourse.tile as tile
from concourse import bacc, mybir
from concourse.bass_utils import run_bass_kernel_spmd

B, S, D = 8, 2048, 512
HID = D // 2
P = 128              # partitions
NS = S // P          # 16 s-tiles
NC = D // P          # 4 d-chunks
NH = HID // P        # 2 hid-chunks
QB = 512             # q-block size
NQB = S // QB        # 4 q-blocks
TPC = QB // P        # 4 s-tiles per block
SCALE = 1.0 / math.sqrt(D)
EXPB = -2.0          # exp bias: keeps exp(s) within fp8e4 range; cancels in O/r
DV = D + 1           # V width incl. ones column
N1 = 256             # PV psum split sizes
N2 = DV - N1         # 257

F32 = mybir.dt.float32
BF16 = mybir.dt.bfloat16
F8 = mybir.dt.float8e4
DR = mybir.MatmulPerfMode.DoubleRow


def build_kernel(gl: float, ga: float, b2val: float):
    nc = bacc.Bacc("TRN2", target_bir_lowering=False, debug=False, num_devices=8)

    aT_d = nc.dram_tensor("aT_d", [NC, P, S], F8, kind="ExternalInput").ap()
    lT_d = nc.dram_tensor("lT_d", [NC, P, S], F8, kind="ExternalInput").ap()
    vT_d = nc.dram_tensor("vT_d", [NC, P, S], BF16, kind="ExternalInput").ap()
    a_d = nc.dram_tensor("a_d", [NS, P, D], BF16, kind="ExternalInput").ap()
    l_d = nc.dram_tensor("l_d", [NS, P, D], BF16, kind="ExternalInput").ap()
    v_d = nc.dram_tensor("v_d", [NS, P, D], BF16, kind="ExternalInput").ap()
    wq_d = nc.dram_tensor("wq_d", [NC, P, D], F8, kind="ExternalInput").ap()
    wk_d = nc.dram_tensor("wk_d", [NC, P, D], F8, kind="ExternalInput").ap()
    wv_d = nc.dram_tensor("wv_d", [NC, P, D], F8, kind="ExternalInput").ap()
    w1_d = nc.dram_tensor("w1_d", [NC, P, HID], BF16, kind="ExternalInput").ap()
    w2_d = nc.dram_tensor("w2_d", [P, NH], BF16, kind="ExternalInput").ap()
    bq_d = nc.dram_tensor("bq_d", [P, NC], F32, kind="ExternalInput").ap()
    bk_d = nc.dram_tensor("bk_d", [P, NC], F32, kind="ExternalInput").ap()
    b1_d = nc.dram_tensor("b1_d", [P, NH], F32, kind="ExternalInput").ap()
    bvgl_d = nc.dram_tensor("bvgl_d", [1, D], F32, kind="ExternalInput").ap()
    out = nc.dram_tensor("out", [NS, P, 3 * D], BF16, kind="ExternalOutput").ap()

    with tile.TileContext(nc) as tc:
        _emit(tc, aT_d, lT_d, vT_d, a_d, l_d, v_d, wq_d, wk_d, wv_d, w1_d,
              w2_d, bq_d, bk_d, b1_d, bvgl_d, out, gl, ga, b2val)

    nc.compile()
    return nc


def _emit(tc, aT_d, lT_d, vT_d, a_d, l_d, v_d, wq_d, wk_d, wv_d, w1_d, w2_d,
          bq_d, bk_d, b1_d, bvgl_d, out, gl, ga, b2val):
    nc = tc.nc
    AF = mybir.ActivationFunctionType
    OP = mybir.AluOpType

    ctx = ExitStack()
    consts = ctx.enter_context(tc.tile_pool(name="consts", bufs=1))
    persist = ctx.enter_context(tc.tile_pool(name="persist", bufs=1))
    stage = ctx.enter_context(tc.tile_pool(name="stage", bufs=2))
    psum_mm = ctx.enter_context(tc.tile_pool(name="psum_mm", bufs=4, space="PSUM"))

    # HAM warm-up: dependency-free matmuls open the PE clock gate (4/8 ->
    # 8/8) while the first DMAs are still streaming in.
    warm_in = consts.tile([P, P], BF16, tag="warm_in")
    nc.vector.memset(warm_in[:], 0.5)
    with tc.tile_pool(name="psum_warm", bufs=1, space="PSUM") as psum_warm:
        wps = psum_warm.tile([P, P], F32, tag="warm")
        for _ in range(144):
            nc.tensor.matmul(
                wps[:], lhsT=warm_in[:], rhs=warm_in[:], start=True, stop=True
            )

    # small parameters on the gpsimd queue (free at start)
    bq_sb = consts.tile([P, NC], F32, tag="bq_sb")
    bk_sb = consts.tile([P, NC], F32, tag="bk_sb")
    b1_sb = consts.tile([P, NH], F32, tag="b1_sb")
    w2_sb = consts.tile([P, NH], BF16, tag="w2_sb")
    nc.gpsimd.dma_start(out=bk_sb[:], in_=bk_d)
    nc.gpsimd.dma_start(out=bq_sb[:], in_=bq_d)
    nc.gpsimd.dma_start(out=b1_sb[:], in_=b1_d)
    nc.gpsimd.dma_start(out=w2_sb[:], in_=w2_d)
    expb_sb = consts.tile([P, 1], F32, tag="expb_sb")   # exp bias constant
    nc.vector.memset(expb_sb[:], EXPB)
    b2h_sb = consts.tile([P, 1], F32, tag="b2h_sb")     # 0.5*b2 for the tanh trick
    nc.vector.memset(b2h_sb[:], 0.5 * b2val)
    bv_bc = consts.tile([P, D], F32, tag="bv_bc")  # gl*bv broadcast to all parts
    bv_bcast_ap = bass.AP(
        tensor=bvgl_d.tensor, offset=bvgl_d.offset, ap=[[0, P], bvgl_d.ap[1]]
    )
    nc.gpsimd.dma_start(out=bv_bc[:], in_=bv_bcast_ap)

    # weights + transposed activations on the sync queue; K's operands first
    wk_sb = consts.tile([P, NC, D], F8, tag="wk_sb")
    wq_sb = consts.tile([P, NC, D], F8, tag="wq_sb")
    wv_sb = consts.tile([P, NC, D], F8, tag="wv_sb")
    w1_sb = consts.tile([P, NC, HID], BF16, tag="w1_sb")
    aT = persist.tile([P, NC, S], F8, tag="aT")
    lT = persist.tile([P, NC, S], F8, tag="lT")
    vT = persist.tile([P, NC, S], BF16, tag="vT")
    nc.sync.dma_start(out=wk_sb[:], in_=wk_d.rearrange("c p d -> p c d"))
    nc.sync.dma_start(out=aT[:], in_=aT_d.rearrange("c p s -> p c s"))
    nc.sync.dma_start(out=wq_sb[:], in_=wq_d.rearrange("c p d -> p c d"))
    nc.sync.dma_start(out=lT[:], in_=lT_d.rearrange("c p s -> p c s"))
    nc.sync.dma_start(out=wv_sb[:], in_=wv_d.rearrange("c p d -> p c d"))
    nc.sync.dma_start(out=w1_sb[:], in_=w1_d.rearrange("c p h -> p c h"))
    nc.sync.dma_start(out=vT[:], in_=vT_d.rearrange("c p s -> p c s"))

    # persistent activations
    kT = persist.tile([P, NC, S], F8, tag="kT")          # K^T [d, s]
    qT = persist.tile([P, NC, S], F8, tag="qT")          # Q^T [d, s]
    v_sb = persist.tile([P, NS, DV], F8, tag="v_sb")     # [gl*V | 1]
    hT = persist.tile([P, NH, S], BF16, tag="hT")        # relu MLP hidden [h, s]
    w_sb = persist.tile([P, NS], F32, tag="w_sb")        # visual weight per s-tile
    nc.vector.memset(v_sb[:, :, D:DV], 1.0)              # ones column

    # ---- streaming phase: projections (fp8 DoubleRow), MLP gate (bf16) ----
    # K^T = Wk^T a^T + bk  (evict on ScalarE), Q^T likewise (evict on DVE)
    for dst, srcT, wgt, bias, on_act in (
        (kT, aT, wk_sb, bk_sb, True),
        (qT, lT, wq_sb, bq_sb, False),
    ):
        for co in range(NC):
            for sb in range(NQB):
                ps = psum_mm.tile([P, QB], F32, tag="mm")
                for j in range(2):
                    nc.tensor.matmul(
                        ps[:],
                        lhsT=wgt[:, 2 * j : 2 * j + 2, co * P : (co + 1) * P],
                        rhs=srcT[:, 2 * j : 2 * j + 2, sb * QB : (sb + 1) * QB],
                        start=(j == 0),
                        stop=(j == 1),
                        perf_mode=DR,
                    )
                dslice = dst[:, co, sb * QB : (sb + 1) * QB]
                if on_act:
                    nc.scalar.activation(
                        out=dslice, in_=ps[:], func=AF.Identity,
                        bias=bias[:, co : co + 1], scale=1.0,
                    )
                else:
                    nc.vector.tensor_scalar(
                        out=dslice, in0=ps[:], scalar1=bias[:, co : co + 1],
                        scalar2=None, op0=OP.add,
                    )

    # V rows (natural [s, d]): v_sb = gl*(a Wv) + gl*bv, straight to fp8
    for st in range(NS):
        ps = psum_mm.tile([P, D], F32, tag="mm")
        for j in range(2):
            nc.tensor.matmul(
                ps[:],
                lhsT=aT[:, 2 * j : 2 * j + 2, st * P : (st + 1) * P],
                rhs=wv_sb[:, 2 * j : 2 * j + 2, :],
                start=(j == 0),
                stop=(j == 1),
                perf_mode=DR,
            )
        nc.vector.scalar_tensor_tensor(
            out=v_sb[:, st, 0:D], in0=ps[:], scalar=gl, in1=bv_bc[:],
            op0=OP.mult, op1=OP.add,
        )

    # fused_a = (1+ga)*a, streamed in natural-layout chunks
    for sc in range(NQB):
        af = stage.tile([P, TPC, D], BF16, tag="a_nat", bufs=2)
        nc.sync.dma_start(
            out=af[:], in_=a_d[sc * TPC : (sc + 1) * TPC].rearrange("t p d -> p t d")
        )
        oa = stage.tile([P, TPC, D], BF16, tag="out_a", bufs=2)
        nc.gpsimd.tensor_scalar_mul(out=oa[:], in0=af[:], scalar1=1.0 + ga)
        nc.scalar.dma_start(
            out=out[sc * TPC : (sc + 1) * TPC, :, D : 2 * D].rearrange("t p d -> p t d"),
            in_=oa[:],
        )

    # hT = relu(W1^T v^T + b1) (bf16), then w = sigmoid(hT^T W2 + b2) via tanh
    with tc.tile_pool(name="psum_w", bufs=2, space="PSUM") as psum_w:
        for ch in range(NH):
            for sb in range(NQB):
                ps = psum_mm.tile([P, QB], F32, tag="mm")
                for ci in range(NC):
                    nc.tensor.matmul(
                        ps[:],
                        lhsT=w1_sb[:, ci, ch * P : (ch + 1) * P],
                        rhs=vT[:, ci, sb * QB : (sb + 1) * QB],
                        start=(ci == 0),
                        stop=(ci == NC - 1),
                    )
                nc.scalar.activation(
                    out=hT[:, ch, sb * QB : (sb + 1) * QB], in_=ps[:],
                    func=AF.Relu, bias=b1_sb[:, ch : ch + 1], scale=1.0,
                )
        for sc in range(NQB):
            for st4 in range(TPC):
                st = sc * TPC + st4
                psw = psum_w.tile([P, 1], F32, tag="small")
                for ch in range(NH):
                    nc.tensor.matmul(
                        psw[:],
                        lhsT=hT[:, ch, st * P : (st + 1) * P],
                        rhs=w2_sb[:, ch : ch + 1],
                        start=(ch == 0),
                        stop=(ch == NH - 1),
                    )
                wt = stage.tile([P, 1], F32, tag="wt", bufs=2)
                nc.scalar.activation(
                    out=wt[:], in_=psw[:], func=AF.Tanh, bias=b2h_sb[:], scale=0.5
                )
                nc.vector.tensor_scalar(
                    out=w_sb[:, st : st + 1], in0=wt[:], scalar1=0.5, scalar2=0.5,
                    op0=OP.mult, op1=OP.add,
                )
            # fused_v = w * v for this chunk (gpsimd + store)
            vf = stage.tile([P, TPC, D], BF16, tag="v_nat", bufs=2)
            nc.sync.dma_start(
                out=vf[:],
                in_=v_d[sc * TPC : (sc + 1) * TPC].rearrange("t p d -> p t d"),
            )
            ov = stage.tile([P, TPC, D], BF16, tag="out_v", bufs=2)
            for st4 in range(TPC):
                st = sc * TPC + st4
                nc.gpsimd.tensor_scalar_mul(
                    out=ov[:, st4, :], in0=vf[:, st4, :],
                    scalar1=w_sb[:, st : st + 1],
                )
            nc.scalar.dma_start(
                out=out[sc * TPC : (sc + 1) * TPC, :, 2 * D : 3 * D].rearrange(
                    "t p d -> p t d"
                ),
                in_=ov[:],
            )

    # ---- attention (fp8 DoubleRow), software-pipelined over q-blocks ----
    with (
        tc.tile_pool(name="ppool", bufs=2) as ppool,
        tc.tile_pool(name="psum_att", bufs=2, space="PSUM") as psum_att,
    ):
        pts = {}

        def emit_scores(qb):
            pT = ppool.tile([P, NS, QB], F8, tag="pT")
            pts[qb] = pT
            for kt in range(NS):
                ps = psum_mm.tile([P, QB], F32, tag="mm")
                for j in range(2):
                    nc.tensor.matmul(
                        ps[:],
                        lhsT=kT[:, 2 * j : 2 * j + 2, kt * P : (kt + 1) * P],
                        rhs=qT[:, 2 * j : 2 * j + 2, qb * QB : (qb + 1) * QB],
                        start=(j == 0),
                        stop=(j == 1),
                        perf_mode=DR,
                    )
                nc.scalar.activation(
                    out=pT[:, kt, :], in_=ps[:], func=AF.Exp, scale=SCALE, bias=expb_sb[:]
                )

        def emit_pv(qb):
            pT = pts.pop(qb)
            lf = stage.tile([P, TPC, D], BF16, tag="l_nat", bufs=2)
            nc.gpsimd.dma_start(
                out=lf[:],
                in_=l_d[qb * TPC : (qb + 1) * TPC].rearrange("t p d -> p t d"),
            )
            lsc = stage.tile([P, TPC, D], F32, tag="lsc", bufs=2)
            nc.gpsimd.tensor_scalar_mul(out=lsc[:], in0=lf[:], scalar1=2.0 - gl)
            ol = stage.tile([P, TPC, D], BF16, tag="out_l", bufs=2)
            for qt in range(TPC):
                o1 = psum_att.tile([P, N1], F32, tag="o1")
                o2 = psum_att.tile([P, N2], F32, tag="o2")
                for t in range(NS // 2):
                    nc.tensor.matmul(
                        o1[:],
                        lhsT=pT[:, 2 * t : 2 * t + 2, qt * P : (qt + 1) * P],
                        rhs=v_sb[:, 2 * t : 2 * t + 2, 0:N1],
                        start=(t == 0),
                        stop=(t == NS // 2 - 1),
                        perf_mode=DR,
                    )
                for t in range(NS // 2):
                    nc.tensor.matmul(
                        o2[:],
                        lhsT=pT[:, 2 * t : 2 * t + 2, qt * P : (qt + 1) * P],
                        rhs=v_sb[:, 2 * t : 2 * t + 2, N1:DV],
                        start=(t == 0),
                        stop=(t == NS // 2 - 1),
                        perf_mode=DR,
                    )
                rinv = stage.tile([P, 1], F32, tag="rinv", bufs=2)
                nc.vector.reciprocal(rinv[:], o2[:, N2 - 1 : N2])
                nc.vector.scalar_tensor_tensor(
                    out=ol[:, qt, 0:N1], in0=o1[:], scalar=rinv[:],
                    in1=lsc[:, qt, 0:N1], op0=OP.mult, op1=OP.add,
                )
                nc.vector.scalar_tensor_tensor(
                    out=ol[:, qt, N1:D], in0=o2[:, 0 : N2 - 1], scalar=rinv[:],
                    in1=lsc[:, qt, N1:D], op0=OP.mult, op1=OP.add,
                )
            nc.scalar.dma_start(
                out=out[qb * TPC : (qb + 1) * TPC, :, 0:D].rearrange("t p d -> p t d"),
                in_=ol[:],
            )

        emit_scores(0)
        emit_scores(1)
        emit_pv(0)
        emit_scores(2)
        emit_pv(1)
        emit_scores(3)
        emit_pv(2)
        emit_pv(3)

    ctx.close()


def _execute(inputs, trace=False, **run_kwargs):
    f8 = ml_dtypes.float8_e4m3
    bf = ml_dtypes.bfloat16
    a = np.ascontiguousarray(np.asarray(inputs["a"], dtype=np.float32))
    v = np.ascontiguousarray(np.asarray(inputs["v"], dtype=np.float32))
    l = np.ascontiguousarray(np.asarray(inputs["l"], dtype=np.float32))
    Wq = np.asarray(inputs["Wq"], dtype=np.float32)
    Wk = np.asarray(inputs["Wk"], dtype=np.float32)
    Wv = np.asarray(inputs["Wv"], dtype=np.float32)
    W1 = np.asarray(inputs["W1"], dtype=np.float32)
    W2 = np.asarray(inputs["W2"], dtype=np.float32)
    bq = np.asarray(inputs["bq"], dtype=np.float32)
    bk = np.asarray(inputs["bk"], dtype=np.float32)
    bv = np.asarray(inputs["bv"], dtype=np.float32)
    b1 = np.asarray(inputs["b1"], dtype=np.float32)
    b2 = np.asarray(inputs["b2"], dtype=np.float32)
    alpha_a = float(np.asarray(inputs["alpha_a"]))
    alpha_l = float(np.asarray(inputs["alpha_l"]))

    gl = float(1.0 / (1.0 + math.exp(-alpha_l)))
    ga = float(1.0 / (1.0 + math.exp(-alpha_a)))
    b2val = float(b2.reshape(-1)[0])

    nc = build_kernel(gl, ga, b2val)

    shared = {
        "wq_d": np.ascontiguousarray(Wq.reshape(NC, P, D)).astype(f8),
        "wk_d": np.ascontiguousarray(Wk.reshape(NC, P, D)).astype(f8),
        "wv_d": np.ascontiguousarray(Wv.reshape(NC, P, D)).astype(f8),
        "w1_d": np.ascontiguousarray(W1.reshape(NC, P, HID)).astype(bf),
        "w2_d": np.ascontiguousarray(W2.reshape(NH, P).T).astype(bf),
        "bq_d": np.ascontiguousarray(bq.reshape(NC, P).T),
        "bk_d": np.ascontiguousarray(bk.reshape(NC, P).T),
        "b1_d": np.ascontiguousarray(b1.reshape(NH, P).T),
        "bvgl_d": np.ascontiguousarray((gl * bv).reshape(1, D)),
    }
    in_maps = []
    for i in range(B):
        m = dict(shared)
        m["aT_d"] = np.ascontiguousarray(a[i].T).astype(f8).reshape(NC, P, S)
        m["lT_d"] = np.ascontiguousarray(l[i].T).astype(f8).reshape(NC, P, S)
        m["vT_d"] = np.ascontiguousarray(v[i].T).astype(bf).reshape(NC, P, S)
        m["a_d"] = a[i].reshape(NS, P, D).astype(bf)
        m["l_d"] = l[i].reshape(NS, P, D).astype(bf)
        m["v_d"] = v[i].reshape(NS, P, D).astype(bf)
        in_maps.append(m)

    res = run_bass_kernel_spmd(
        nc, in_maps, core_ids=list(range(B)), trace=trace, **run_kwargs
    )
    outs = [
        res.results[i]["out"].astype(np.float32).reshape(S, 3 * D) for i in range(B)
    ]
    return np.stack(outs, axis=0), res


def kernel(**inputs) -> np.ndarray:
    out, _ = _execute(inputs, trace=False)
    return out


if __name__ == "__main__":
    print("kernel module OK")


# revision 5
# speedup vs baseline: 2.6920x; 2.6920x over previous
"""CrossAttentionFusion kernel for Trainium2 (8 NeuronCores, data-parallel over batch).

Reference computation (per batch element, S=2048, D=512, HID=256):
  Q = l @ Wq + bq ; K = a @ Wk + bk ; V = a @ Wv + bv
  P = softmax(Q K^T / sqrt(D)) ; O = P @ V
  fused_l = gl*O + (2-gl)*l          (gl = sigmoid(alpha_l))
  fused_a = (1+ga)*a                 (ga = sigmoid(alpha_a))
  w = sigmoid(relu(v @ W1 + b1) @ W2 + b2) ; fused_v = w*v
  out = concat([fused_l, fused_a, fused_v], -1)     # [S, 3D]

Kernel strategy (per core, one batch element):
  - the host feeds activations in matmul-ready layouts: a^T/l^T in fp8e4 and
    v^T in bf16 ([d, s], chunked over 128-partition d-slices), plus natural
    bf16 copies for the elementwise epilogues. This removes all on-chip
    transposes and dtype-cast passes.
  - Q/K/V projections and both attention matmuls run in fp8e4 with DoubleRow
    perf mode (two 128-row k-subtiles per matmul -> 2x PE throughput); the
    visual-gate MLP stays bf16. All accumulation is fp32 in PSUM.
    Measured end-to-end rel err ~3e-3 (validated against a numpy simulation
    of the exact quantization points).
  - softmax skips the max pass: P = exp(s/sqrt(D) - 2) written straight to
    fp8 (the -2 bias keeps exp() below the fp8e4 max of 240; the constant
    cancels in the rowsum normalization). The rowsum comes from a ones
    column appended to V (PSUM split 256+257 to stay within banks).
  - gl is folded into V's bias-add eviction (host pre-scales bv by gl), so
    P@V already yields gl*O and the epilogue is (O_acc * (1/r)) + (2-gl)*l
    in one DVE pass per half.
  - attention is software-pipelined: scores(qb+1) is emitted between
    scores(qb) and PV(qb) so the PE never waits on the ScalarE Exp stream.
"""

import math
from contextlib import ExitStack

import ml_dtypes
import numpy as np

import concourse.bass as bass
import concourse.tile as tile
from concourse import bacc, mybir
from concourse.bass_utils import run_bass_kernel_spmd

B, S, D = 8, 2048, 512
HID = D // 2
P = 128              # partitions
NS = S // P          # 16 s-tiles
NC = D // P          # 4 d-chunks
NH = HID // P        # 2 hid-chunks
QB = 512             # q-block size
NQB = S // QB        # 4 q-blocks
TPC = QB // P        # 4 s-tiles per block
SCALE = 1.0 / math.sqrt(D)
EXPB = -2.0          # exp bias: keeps exp(s) within fp8e4 range; cancels in O/r
DV = D + 1           # V width incl. ones column
N1 = 256             # PV psum split sizes
N2 = DV - N1         # 257

F32 = mybir.dt.float32
BF16 = mybir.dt.bfloat16
F8 = mybir.dt.float8e4
DR = mybir.MatmulPerfMode.DoubleRow


def build_kernel(gl: float, ga: float, b2val: float):
    nc = bacc.Bacc("TRN2", target_bir_lowering=False, debug=False, num_devices=8)

    aT_d = nc.dram_tensor("aT_d", [NC, P, S], F8, kind="ExternalInput").ap()
    lT_d = nc.dram_tensor("lT_d", [NC, P, S], F8, kind="ExternalInput").ap()
    vT_d = nc.dram_tensor("vT_d", [NC, P, S], BF16, kind="ExternalInput").ap()
    a_d = nc.dram_tensor("a_d", [NS, P, D], BF16, kind="ExternalInput").ap()
    l_d = nc.dram_tensor("l_d", [NS, P, D], BF16, kind="ExternalInput").ap()
    v_d = nc.dram_tensor("v_d", [NS, P, D], BF16, kind="ExternalInput").ap()
    wq_d = nc.dram_tensor("wq_d", [NC, P, D], F8, kind="ExternalInput").ap()
    wk_d = nc.dram_tensor("wk_d", [NC, P, D], F8, kind="ExternalInput").ap()
    wv_d = nc.dram_tensor("wv_d", [NC, P, D], F8, kind="ExternalInput").ap()
    w1_d = nc.dram_tensor("w1_d", [NC, P, HID], BF16, kind="ExternalInput").ap()
    w2_d = nc.dram_tensor("w2_d", [P, NH], BF16, kind="ExternalInput").ap()
    bq_d = nc.dram_tensor("bq_d", [P, NC], F32, kind="ExternalInput").ap()
    bk_d = nc.dram_tensor("bk_d", [P, NC], F32, kind="ExternalInput").ap()
    b1_d = nc.dram_tensor("b1_d", [P, NH], F32, kind="ExternalInput").ap()
    bvgl_d = nc.dram_tensor("bvgl_d", [1, D], F32, kind="ExternalInput").ap()
    out = nc.dram_tensor("out", [NS, P, 3 * D], BF16, kind="ExternalOutput").ap()

    with tile.TileContext(nc) as tc:
        _emit(tc, aT_d, lT_d, vT_d, a_d, l_d, v_d, wq_d, wk_d, wv_d, w1_d,
              w2_d, bq_d, bk_d, b1_d, bvgl_d, out, gl, ga, b2val)

    nc.compile()
    return nc


def _emit(tc, aT_d, lT_d, vT_d, a_d, l_d, v_d, wq_d, wk_d, wv_d, w1_d, w2_d,
          bq_d, bk_d, b1_d, bvgl_d, out, gl, ga, b2val):
    nc = tc.nc
    AF = mybir.ActivationFunctionType
    OP = mybir.AluOpType

    ctx = ExitStack()
    consts = ctx.enter_context(tc.tile_pool(name="consts", bufs=1))
    persist = ctx.enter_context(tc.tile_pool(name="persist", bufs=1))
    stage = ctx.enter_context(tc.tile_pool(name="stage", bufs=2))
    psum_mm = ctx.enter_context(tc.tile_pool(name="psum_mm", bufs=4, space="PSUM"))

    # HAM warm-up: dependency-free matmuls open the PE clock gate (4/8 ->
    # 8/8) while the first DMAs are still streaming in.
    warm_in = consts.tile([P, P], BF16, tag="warm_in")
    nc.vector.memset(warm_in[:], 0.5)
    with tc.tile_pool(name="psum_warm", bufs=1, space="PSUM") as psum_warm:
        wps = psum_warm.tile([P, P], F32, tag="warm")
        for _ in range(144):
            nc.tensor.matmul(
                wps[:], lhsT=warm_in[:], rhs=warm_in[:], start=True, stop=True
            )

    # small parameters on the gpsimd queue (free at start)
    bq_sb = consts.tile([P, NC], F32, tag="bq_sb")
    bk_sb = consts.tile([P, NC], F32, tag="bk_sb")
    b1_sb = consts.tile([P, NH], F32, tag="b1_sb")
    w2_sb = consts.tile([P, NH], BF16, tag="w2_sb")
    nc.gpsimd.dma_start(out=bk_sb[:], in_=bk_d)
    nc.gpsimd.dma_start(out=bq_sb[:], in_=bq_d)
    nc.gpsimd.dma_start(out=b1_sb[:], in_=b1_d)
    nc.gpsimd.dma_start(out=w2_sb[:], in_=w2_d)
    expb_sb = consts.tile([P, 1], F32, tag="expb_sb")   # exp bias constant
    nc.vector.memset(expb_sb[:], EXPB)
    b2h_sb = consts.tile([P, 1], F32, tag="b2h_sb")     # 0.5*b2 for the tanh trick
    nc.vector.memset(b2h_sb[:], 0.5 * b2val)
    bv_bc = consts.tile([P, D], F32, tag="bv_bc")  # gl*bv broadcast to all parts
    bv_bcast_ap = bass.AP(
        tensor=bvgl_d.tensor, offset=bvgl_d.offset, ap=[[0, P], bvgl_d.ap[1]]
    )
    nc.gpsimd.dma_start(out=bv_bc[:], in_=bv_bcast_ap)

    # weights + transposed activations on the sync queue; K's operands first
    wk_sb = consts.tile([P, NC, D], F8, tag="wk_sb")
    wq_sb = consts.tile([P, NC, D], F8, tag="wq_sb")
    wv_sb = consts.tile([P, NC, D], F8, tag="wv_sb")
    w1_sb = consts.tile([P, NC, HID], BF16, tag="w1_sb")
    aT = persist.tile([P, NC, S], F8, tag="aT")
    lT = persist.tile([P, NC, S], F8, tag="lT")
    vT = persist.tile([P, NC, S], BF16, tag="vT")
    nc.sync.dma_start(out=wk_sb[:], in_=wk_d.rearrange("c p d -> p c d"))
    nc.sync.dma_start(out=aT[:], in_=aT_d.rearrange("c p s -> p c s"))
    nc.sync.dma_start(out=wq_sb[:], in_=wq_d.rearrange("c p d -> p c d"))
    nc.sync.dma_start(out=lT[:], in_=lT_d.rearrange("c p s -> p c s"))
    nc.sync.dma_start(out=wv_sb[:], in_=wv_d.rearrange("c p d -> p c d"))
    nc.sync.dma_start(out=w1_sb[:], in_=w1_d.rearrange("c p h -> p c h"))
    nc.sync.dma_start(out=vT[:], in_=vT_d.rearrange("c p s -> p c s"))

    # persistent activations
    kT = persist.tile([P, NC, S], F8, tag="kT")          # K^T [d, s]
    qT = persist.tile([P, NC, S], F8, tag="qT")          # Q^T [d, s]
    v_sb = persist.tile([P, NS, DV], F8, tag="v_sb")     # [gl*V | 1]
    hT = persist.tile([P, NH, S], BF16, tag="hT")        # relu MLP hidden [h, s]
    w_sb = persist.tile([P, NS], F32, tag="w_sb")        # visual weight per s-tile
    nc.vector.memset(v_sb[:, :, D:DV], 1.0)              # ones column

    # ---- streaming phase: projections (fp8 DoubleRow), MLP gate (bf16) ----
    # K^T = Wk^T a^T + bk  (evict on ScalarE), Q^T likewise (evict on DVE)
    for dst, srcT, wgt, bias, on_act in (
        (kT, aT, wk_sb, bk_sb, True),
        (qT, lT, wq_sb, bq_sb, False),
    ):
        for co in range(NC):
            for sb in range(NQB):
                ps = psum_mm.tile([P, QB], F32, tag="mm")
                for j in range(2):
                    nc.tensor.matmul(
                        ps[:],
                        lhsT=wgt[:, 2 * j : 2 * j + 2, co * P : (co + 1) * P],
                        rhs=srcT[:, 2 * j : 2 * j + 2, sb * QB : (sb + 1) * QB],
                        start=(j == 0),
                        stop=(j == 1),
                        perf_mode=DR,
                    )
                dslice = dst[:, co, sb * QB : (sb + 1) * QB]
                if on_act:
                    nc.scalar.activation(
                        out=dslice, in_=ps[:], func=AF.Identity,
                        bias=bias[:, co : co + 1], scale=1.0,
                    )
                else:
                    nc.vector.tensor_scalar(
                        out=dslice, in0=ps[:], scalar1=bias[:, co : co + 1],
                        scalar2=None, op0=OP.add,
                    )

    # V rows (natural [s, d]): v_sb = gl*(a Wv) + gl*bv, straight to fp8
    for st in range(NS):
        ps = psum_mm.tile([P, D], F32, tag="mm")
        for j in range(2):
            nc.tensor.matmul(
                ps[:],
                lhsT=aT[:, 2 * j : 2 * j + 2, st * P : (st + 1) * P],
                rhs=wv_sb[:, 2 * j : 2 * j + 2, :],
                start=(j == 0),
                stop=(j == 1),
                perf_mode=DR,
            )
        nc.vector.scalar_tensor_tensor(
            out=v_sb[:, st, 0:D], in0=ps[:], scalar=gl, in1=bv_bc[:],
            op0=OP.mult, op1=OP.add,
        )

    # fused_a = (1+ga)*a, streamed in natural-layout chunks
    for sc in range(NQB):
        af = stage.tile([P, TPC, D], BF16, tag="a_nat", bufs=2)
        nc.sync.dma_start(
            out=af[:], in_=a_d[sc * TPC : (sc + 1) * TPC].rearrange("t p d -> p t d")
        )
        oa = stage.tile([P, TPC, D], BF16, tag="out_a", bufs=2)
        nc.vector.tensor_scalar_mul(out=oa[:], in0=af[:], scalar1=1.0 + ga)
        nc.scalar.dma_start(
            out=out[sc * TPC : (sc + 1) * TPC, :, D : 2 * D].rearrange("t p d -> p t d"),
            in_=oa[:],
        )

    # hT = relu(W1^T v^T + b1) (bf16), then w = sigmoid(hT^T W2 + b2) via tanh
    with tc.tile_pool(name="psum_w", bufs=2, space="PSUM") as psum_w:
        for ch in range(NH):
            for sb in range(NQB):
                ps = psum_mm.tile([P, QB], F32, tag="mm")
                for ci in range(NC):
                    nc.tensor.matmul(
                        ps[:],
                        lhsT=w1_sb[:, ci, ch * P : (ch + 1) * P],
                        rhs=vT[:, ci, sb * QB : (sb + 1) * QB],
                        start=(ci == 0),
                        stop=(ci == NC - 1),
                    )
                nc.scalar.activation(
                    out=hT[:, ch, sb * QB : (sb + 1) * QB], in_=ps[:],
                    func=AF.Relu, bias=b1_sb[:, ch : ch + 1], scale=1.0,
                )
        for sc in range(NQB):
            for st4 in range(TPC):
                st = sc * TPC + st4
                psw = psum_w.tile([P, 1], F32, tag="small")
                for ch in range(NH):
                    nc.tensor.matmul(
                        psw[:],
                        lhsT=hT[:, ch, st * P : (st + 1) * P],
                        rhs=w2_sb[:, ch : ch + 1],
                        start=(ch == 0),
                        stop=(ch == NH - 1),
                    )
                wt = stage.tile([P, 1], F32, tag="wt", bufs=2)
                nc.scalar.activation(
                    out=wt[:], in_=psw[:], func=AF.Tanh, bias=b2h_sb[:], scale=0.5
                )
                nc.vector.tensor_scalar(
                    out=w_sb[:, st : st + 1], in0=wt[:], scalar1=0.5, scalar2=0.5,
                    op0=OP.mult, op1=OP.add,
                )
            # fused_v = w * v for this chunk (gpsimd + store)
            vf = stage.tile([P, TPC, D], BF16, tag="v_nat", bufs=2)
            nc.sync.dma_start(
                out=vf[:],
                in_=v_d[sc * TPC : (sc + 1) * TPC].rearrange("t p d -> p t d"),
            )
            ov = stage.tile([P, TPC, D], BF16, tag="out_v", bufs=2)
            for st4 in range(TPC):
                st = sc * TPC + st4
                nc.vector.tensor_scalar_mul(
                    out=ov[:, st4, :], in0=vf[:, st4, :],
                    scalar1=w_sb[:, st : st + 1],
                )
            nc.scalar.dma_start(
                out=out[sc * TPC : (sc + 1) * TPC, :, 2 * D : 3 * D].rearrange(
                    "t p d -> p t d"
                ),
                in_=ov[:],
            )

    # ---- attention (fp8 DoubleRow), software-pipelined over q-blocks ----
    with (
        tc.tile_pool(name="ppool", bufs=2) as ppool,
        tc.tile_pool(name="psum_att", bufs=2, space="PSUM") as psum_att,
    ):
        pts = {}

        def emit_scores(qb):
            pT = ppool.tile([P, NS, QB], F8, tag="pT")
            pts[qb] = pT
            for kt in range(NS):
                ps = psum_mm.tile([P, QB], F32, tag="mm")
                for j in range(2):
                    nc.tensor.matmul(
                        ps[:],
                        lhsT=kT[:, 2 * j : 2 * j + 2, kt * P : (kt + 1) * P],
                        rhs=qT[:, 2 * j : 2 * j + 2, qb * QB : (qb + 1) * QB],
                        start=(j == 0),
                        stop=(j == 1),
                        perf_mode=DR,
                    )
                nc.scalar.activation(
                    out=pT[:, kt, :], in_=ps[:], func=AF.Exp, scale=SCALE, bias=expb_sb[:]
                )

        def emit_pv(qb):
            pT = pts.pop(qb)
            lf = stage.tile([P, TPC, D], BF16, tag="l_nat", bufs=2)
            nc.gpsimd.dma_start(
                out=lf[:],
                in_=l_d[qb * TPC : (qb + 1) * TPC].rearrange("t p d -> p t d"),
            )
            lsc = stage.tile([P, TPC, D], F32, tag="lsc", bufs=2)
            nc.scalar.mul(lsc[:], lf[:], 2.0 - gl)
            ol = stage.tile([P, TPC, D], BF16, tag="out_l", bufs=2)
            for qt in range(TPC):
                o1 = psum_att.tile([P, N1], F32, tag="o1")
                o2 = psum_att.tile([P, N2], F32, tag="o2")
                for t in range(NS // 2):
                    nc.tensor.matmul(
                        o1[:],
                        lhsT=pT[:, 2 * t : 2 * t + 2, qt * P : (qt + 1) * P],
                        rhs=v_sb[:, 2 * t : 2 * t + 2, 0:N1],
                        start=(t == 0),
                        stop=(t == NS // 2 - 1),
                        perf_mode=DR,
                    )
                for t in range(NS // 2):
                    nc.tensor.matmul(
                        o2[:],
                        lhsT=pT[:, 2 * t : 2 * t + 2, qt * P : (qt + 1) * P],
                        rhs=v_sb[:, 2 * t : 2 * t + 2, N1:DV],
                        start=(t == 0),
                        stop=(t == NS // 2 - 1),
                        perf_mode=DR,
                    )
                rinv = stage.tile([P, 1], F32, tag="rinv", bufs=2)
                nc.vector.reciprocal(rinv[:], o2[:, N2 - 1 : N2])
                nc.vector.scalar_tensor_tensor(
                    out=ol[:, qt, 0:N1], in0=o1[:], scalar=rinv[:],
                    in1=lsc[:, qt, 0:N1], op0=OP.mult, op1=OP.add,
                )
                nc.vector.scalar_tensor_tensor(
                    out=ol[:, qt, N1:D], in0=o2[:, 0 : N2 - 1], scalar=rinv[:],
                    in1=lsc[:, qt, N1:D], op0=OP.mult, op1=OP.add,
                )
            nc.scalar.dma_start(
                out=out[qb * TPC : (qb + 1) * TPC, :, 0:D].rearrange("t p d -> p t d"),
                in_=ol[:],
            )

        emit_scores(0)
        emit_scores(1)
        emit_pv(0)
        emit_scores(2)
        emit_pv(1)
        emit_scores(3)
        emit_pv(2)
        emit_pv(3)

    ctx.close()


def _execute(inputs, trace=False, **run_kwargs):
    f8 = ml_dtypes.float8_e4m3
    bf = ml_dtypes.bfloat16
    a = np.ascontiguousarray(np.asarray(inputs["a"], dtype=np.float32))
    v = np.ascontiguousarray(np.asarray(inputs["v"], dtype=np.float32))
    l = np.ascontiguousarray(np.asarray(inputs["l"], dtype=np.float32))
    Wq = np.asarray(inputs["Wq"], dtype=np.float32)
    Wk = np.asarray(inputs["Wk"], dtype=np.float32)
    Wv = np.asarray(inputs["Wv"], dtype=np.float32)
    W1 = np.asarray(inputs["W1"], dtype=np.float32)
    W2 = np.asarray(inputs["W2"], dtype=np.float32)
    bq = np.asarray(inputs["bq"], dtype=np.float32)
    bk = np.asarray(inputs["bk"], dtype=np.float32)
    bv = np.asarray(inputs["bv"], dtype=np.float32)
    b1 = np.asarray(inputs["b1"], dtype=np.float32)
    b2 = np.asarray(inputs["b2"], dtype=np.float32)
    alpha_a = float(np.asarray(inputs["alpha_a"]))
    alpha_l = float(np.asarray(inputs["alpha_l"]))

    gl = float(1.0 / (1.0 + math.exp(-alpha_l)))
    ga = float(1.0 / (1.0 + math.exp(-alpha_a)))
    b2val = float(b2.reshape(-1)[0])

    nc = build_kernel(gl, ga, b2val)

    shared = {
        "wq_d": np.ascontiguousarray(Wq.reshape(NC, P, D)).astype(f8),
        "wk_d": np.ascontiguousarray(Wk.reshape(NC, P, D)).astype(f8),
        "wv_d": np.ascontiguousarray(Wv.reshape(NC, P, D)).astype(f8),
        "w1_d": np.ascontiguousarray(W1.reshape(NC, P, HID)).astype(bf),
        "w2_d": np.ascontiguousarray(W2.reshape(NH, P).T).astype(bf),
        "bq_d": np.ascontiguousarray(bq.reshape(NC, P).T),
        "bk_d": np.ascontiguousarray(bk.reshape(NC, P).T),
        "b1_d": np.ascontiguousarray(b1.reshape(NH, P).T),
        "bvgl_d": np.ascontiguousarray((gl * bv).reshape(1, D)),
    }
    in_maps = []
    for i in range(B):
        m = dict(shared)
        m["aT_d"] = np.ascontiguousarray(a[i].T).astype(f8).reshape(NC, P, S)
        m["lT_d"] = np.ascontiguousarray(l[i].T).astype(f8).reshape(NC, P, S)
        m["vT_d"] = np.ascontiguousarray(v[i].T).astype(bf).reshape(NC, P, S)
        m["a_d"] = a[i].reshape(NS, P, D).astype(bf)
        m["l_d"] = l[i].reshape(NS, P, D).astype(bf)
        m["v_d"] = v[i].reshape(NS, P, D).astype(bf)
        in_maps.append(m)

    res = run_bass_kernel_spmd(
        nc, in_maps, core_ids=list(range(B)), trace=trace, **run_kwargs
    )
    outs = [
        res.results[i]["out"].astype(np.float32).reshape(S, 3 * D) for i in range(B)
    ]
    return np.stack(outs, axis=0), res


def kernel(**inputs) -> np.ndarray:
    out, _ = _execute(inputs, trace=False)
    return out


if __name__ == "__main__":
    print("kernel module OK")


# revision 6
# speedup vs baseline: 3.0447x; 1.1310x over previous
"""CrossAttentionFusion kernel for Trainium2 (8 NeuronCores, data-parallel over batch).

Reference computation (per batch element, S=2048, D=512, HID=256):
  Q = l @ Wq + bq ; K = a @ Wk + bk ; V = a @ Wv + bv
  P = softmax(Q K^T / sqrt(D)) ; O = P @ V
  fused_l = gl*O + (2-gl)*l          (gl = sigmoid(alpha_l))
  fused_a = (1+ga)*a                 (ga = sigmoid(alpha_a))
  w = sigmoid(relu(v @ W1 + b1) @ W2 + b2) ; fused_v = w*v
  out = concat([fused_l, fused_a, fused_v], -1)     # [S, 3D]

Kernel strategy (per core, one batch element):
  - the host feeds activations in matmul-ready layouts: a^T/l^T in fp8e4 and
    v^T in bf16 ([d, s], chunked over 128-partition d-slices), plus natural
    bf16 copies for the elementwise epilogues. This removes all on-chip
    transposes and dtype-cast passes.
  - Q/K/V projections and both attention matmuls run in fp8e4 with DoubleRow
    perf mode (two 128-row k-subtiles per matmul -> 2x PE throughput); the
    visual-gate MLP stays bf16. All accumulation is fp32 in PSUM.
    Measured end-to-end rel err ~3e-3 (validated against a numpy simulation
    of the exact quantization points).
  - softmax skips the max pass: P = exp(s/sqrt(D) - 2) written straight to
    fp8 (the -2 bias keeps exp() below the fp8e4 max of 240; the constant
    cancels in the rowsum normalization). The rowsum comes from a ones
    column appended to V (PSUM split 256+257 to stay within banks).
  - gl is folded into V's eviction scale, so P@V already yields gl*O; bv is
    applied exactly at the epilogue via the rowsum identity
    P@(V+bv 1^T)/r = P@V/r + bv. The epilogue is one DVE pass per half:
    (O_acc * (1/r)) + [(2-gl)*l + gl*bv].
  - attention is software-pipelined: scores(qb+1) is emitted between
    scores(qb) and PV(qb) so the PE never waits on the ScalarE Exp stream.
"""

import math
from contextlib import ExitStack

import ml_dtypes
import numpy as np

import concourse.bass as bass
import concourse.tile as tile
from concourse import bacc, mybir
from concourse.bass_utils import run_bass_kernel_spmd

B, S, D = 8, 2048, 512
HID = D // 2
P = 128              # partitions
NS = S // P          # 16 s-tiles
NC = D // P          # 4 d-chunks
NH = HID // P        # 2 hid-chunks
QB = 512             # q-block size
NQB = S // QB        # 4 q-blocks
TPC = QB // P        # 4 s-tiles per block
SCALE = 1.0 / math.sqrt(D)
EXPB = -2.0          # exp bias: keeps exp(s) within fp8e4 range; cancels in O/r
DV = D + 1           # V width incl. ones column
N1 = 256             # PV psum split sizes
N2 = DV - N1         # 257

F32 = mybir.dt.float32
BF16 = mybir.dt.bfloat16
F8 = mybir.dt.float8e4
DR = mybir.MatmulPerfMode.DoubleRow


def build_kernel(gl: float, ga: float, b2val: float):
    nc = bacc.Bacc("TRN2", target_bir_lowering=False, debug=False, num_devices=8)

    aT_d = nc.dram_tensor("aT_d", [NC, P, S], F8, kind="ExternalInput").ap()
    lT_d = nc.dram_tensor("lT_d", [NC, P, S], F8, kind="ExternalInput").ap()
    vT_d = nc.dram_tensor("vT_d", [NC, P, S], BF16, kind="ExternalInput").ap()
    a_d = nc.dram_tensor("a_d", [NS, P, D], BF16, kind="ExternalInput").ap()
    l_d = nc.dram_tensor("l_d", [NS, P, D], BF16, kind="ExternalInput").ap()
    v_d = nc.dram_tensor("v_d", [NS, P, D], BF16, kind="ExternalInput").ap()
    wq_d = nc.dram_tensor("wq_d", [NC, P, D], F8, kind="ExternalInput").ap()
    wk_d = nc.dram_tensor("wk_d", [NC, P, D], F8, kind="ExternalInput").ap()
    wv_d = nc.dram_tensor("wv_d", [NC, P, D], F8, kind="ExternalInput").ap()
    w1_d = nc.dram_tensor("w1_d", [NC, P, HID], BF16, kind="ExternalInput").ap()
    w2_d = nc.dram_tensor("w2_d", [P, NH], BF16, kind="ExternalInput").ap()
    bq_d = nc.dram_tensor("bq_d", [P, NC], F32, kind="ExternalInput").ap()
    bk_d = nc.dram_tensor("bk_d", [P, NC], F32, kind="ExternalInput").ap()
    b1_d = nc.dram_tensor("b1_d", [P, NH], F32, kind="ExternalInput").ap()
    bvgl_d = nc.dram_tensor("bvgl_d", [1, D], F32, kind="ExternalInput").ap()
    out = nc.dram_tensor("out", [NS, P, 3 * D], BF16, kind="ExternalOutput").ap()

    with tile.TileContext(nc) as tc:
        _emit(tc, aT_d, lT_d, vT_d, a_d, l_d, v_d, wq_d, wk_d, wv_d, w1_d,
              w2_d, bq_d, bk_d, b1_d, bvgl_d, out, gl, ga, b2val)

    nc.compile()
    return nc


def _emit(tc, aT_d, lT_d, vT_d, a_d, l_d, v_d, wq_d, wk_d, wv_d, w1_d, w2_d,
          bq_d, bk_d, b1_d, bvgl_d, out, gl, ga, b2val):
    nc = tc.nc
    AF = mybir.ActivationFunctionType
    OP = mybir.AluOpType

    ctx = ExitStack()
    consts = ctx.enter_context(tc.tile_pool(name="consts", bufs=1))
    persist = ctx.enter_context(tc.tile_pool(name="persist", bufs=1))
    stage = ctx.enter_context(tc.tile_pool(name="stage", bufs=2))

    # HAM warm-up: dependency-free matmuls open the PE clock gate (4/8 ->
    # 8/8) while the first DMAs are still streaming in.
    warm_in = consts.tile([P, P], BF16, tag="warm_in")
    nc.vector.memset(warm_in[:], 0.5)
    with tc.tile_pool(name="psum_warm", bufs=1, space="PSUM") as psum_warm:
        wps = psum_warm.tile([P, P], F32, tag="warm")
        for _ in range(56):
            nc.tensor.matmul(
                wps[:], lhsT=warm_in[:], rhs=warm_in[:], start=True, stop=True
            )

    # small parameters on the gpsimd queue (free at start)
    bq_sb = consts.tile([P, NC], F32, tag="bq_sb")
    bk_sb = consts.tile([P, NC], F32, tag="bk_sb")
    b1_sb = consts.tile([P, NH], F32, tag="b1_sb")
    w2_sb = consts.tile([P, NH], BF16, tag="w2_sb")
    nc.gpsimd.dma_start(out=bk_sb[:], in_=bk_d)
    nc.gpsimd.dma_start(out=bq_sb[:], in_=bq_d)
    nc.gpsimd.dma_start(out=b1_sb[:], in_=b1_d)
    nc.gpsimd.dma_start(out=w2_sb[:], in_=w2_d)
    expb_sb = consts.tile([P, 1], F32, tag="expb_sb")   # exp bias constant
    nc.vector.memset(expb_sb[:], EXPB)
    b2h_sb = consts.tile([P, 1], F32, tag="b2h_sb")     # 0.5*b2 for the tanh trick
    nc.vector.memset(b2h_sb[:], 0.5 * b2val)
    bv_bc = consts.tile([P, D], F32, tag="bv_bc")  # gl*bv broadcast to all parts
    bv_bcast_ap = bass.AP(
        tensor=bvgl_d.tensor, offset=bvgl_d.offset, ap=[[0, P], bvgl_d.ap[1]]
    )
    nc.gpsimd.dma_start(out=bv_bc[:], in_=bv_bcast_ap)

    # big loads split over two HWDGE queues; K's operands lead each queue
    wk_sb = consts.tile([P, NC, D], F8, tag="wk_sb")
    wq_sb = consts.tile([P, NC, D], F8, tag="wq_sb")
    wv_sb = consts.tile([P, NC, D], F8, tag="wv_sb")
    w1_sb = consts.tile([P, NC, HID], BF16, tag="w1_sb")
    aT = persist.tile([P, NC, S], F8, tag="aT")
    lT = persist.tile([P, NC, S], F8, tag="lT")
    vT = persist.tile([P, NC, S], BF16, tag="vT")
    nc.sync.dma_start(out=wk_sb[:], in_=wk_d.rearrange("c p d -> p c d"))
    nc.sync.dma_start(out=aT[:], in_=aT_d.rearrange("c p s -> p c s"))
    nc.sync.dma_start(out=wq_sb[:], in_=wq_d.rearrange("c p d -> p c d"))
    nc.sync.dma_start(out=lT[:], in_=lT_d.rearrange("c p s -> p c s"))
    nc.scalar.dma_start(out=wv_sb[:], in_=wv_d.rearrange("c p d -> p c d"))
    nc.scalar.dma_start(out=w1_sb[:], in_=w1_d.rearrange("c p h -> p c h"))
    nc.scalar.dma_start(out=vT[:], in_=vT_d.rearrange("c p s -> p c s"))

    # persistent activations
    kT = persist.tile([P, NC, S], F8, tag="kT")          # K^T [d, s]
    qT = persist.tile([P, NC, S], F8, tag="qT")          # Q^T [d, s]
    v_sb = persist.tile([P, NS, DV], F8, tag="v_sb")     # [gl*V | 1]
    hT = persist.tile([P, NH, S], BF16, tag="hT")        # relu MLP hidden [h, s]
    w_sb = persist.tile([P, NS], F32, tag="w_sb")        # visual weight per s-tile
    nc.vector.memset(v_sb[:, :, D:DV], 1.0)              # ones column

    # ---- streaming phase: projections (fp8 DoubleRow), MLP gate (bf16) ----
    with (
        tc.tile_pool(name="psum_mm", bufs=4, space="PSUM") as psum_mm,
        tc.tile_pool(name="psum_w", bufs=2, space="PSUM") as psum_w,
    ):
        # K^T = Wk^T a^T + bk (evict on ScalarE), Q^T likewise (evict on DVE)
        for dst, srcT, wgt, bias, on_act in (
            (kT, aT, wk_sb, bk_sb, True),
            (qT, lT, wq_sb, bq_sb, False),
        ):
            for co in range(NC):
                for sb in range(NQB):
                    ps = psum_mm.tile([P, QB], F32, tag="mm")
                    for j in range(2):
                        nc.tensor.matmul(
                            ps[:],
                            lhsT=wgt[:, 2 * j : 2 * j + 2, co * P : (co + 1) * P],
                            rhs=srcT[:, 2 * j : 2 * j + 2, sb * QB : (sb + 1) * QB],
                            start=(j == 0),
                            stop=(j == 1),
                            perf_mode=DR,
                        )
                    dslice = dst[:, co, sb * QB : (sb + 1) * QB]
                    if on_act:
                        nc.scalar.activation(
                            out=dslice, in_=ps[:], func=AF.Identity,
                            bias=bias[:, co : co + 1], scale=1.0,
                        )
                    else:
                        nc.vector.tensor_scalar(
                            out=dslice, in0=ps[:], scalar1=bias[:, co : co + 1],
                            scalar2=None, op0=OP.add,
                        )

        # V rows (natural [s, d]): v_sb = gl*(a Wv), straight to fp8. bv is
        # applied exactly in the fused_l epilogue via the rowsum identity.
        # Evictions alternate ScalarE/DVE to balance the engines.
        for st in range(NS):
            ps = psum_mm.tile([P, D], F32, tag="mm")
            for j in range(2):
                nc.tensor.matmul(
                    ps[:],
                    lhsT=aT[:, 2 * j : 2 * j + 2, st * P : (st + 1) * P],
                    rhs=wv_sb[:, 2 * j : 2 * j + 2, :],
                    start=(j == 0),
                    stop=(j == 1),
                    perf_mode=DR,
                )
            if st % 2 == 0:
                nc.scalar.mul(v_sb[:, st, 0:D], ps[:], gl)
            else:
                nc.vector.tensor_scalar_mul(
                    out=v_sb[:, st, 0:D], in0=ps[:], scalar1=gl
                )

        # fused_a = (1+ga)*a, streamed in natural-layout chunks
        for sc in range(NQB):
            af = stage.tile([P, TPC, D], BF16, tag="a_nat", bufs=2)
            nc.sync.dma_start(
                out=af[:],
                in_=a_d[sc * TPC : (sc + 1) * TPC].rearrange("t p d -> p t d"),
            )
            oa = stage.tile([P, TPC, D], BF16, tag="out_a", bufs=2)
            nc.vector.tensor_scalar_mul(out=oa[:], in0=af[:], scalar1=1.0 + ga)
            nc.scalar.dma_start(
                out=out[sc * TPC : (sc + 1) * TPC, :, D : 2 * D].rearrange(
                    "t p d -> p t d"
                ),
                in_=oa[:],
            )

        # hT = relu(W1^T v^T + b1) (bf16), then w = sigmoid(hT^T W2 + b2)
        for ch in range(NH):
            for sb in range(NQB):
                ps = psum_mm.tile([P, QB], F32, tag="mm")
                for ci in range(NC):
                    nc.tensor.matmul(
                        ps[:],
                        lhsT=w1_sb[:, ci, ch * P : (ch + 1) * P],
                        rhs=vT[:, ci, sb * QB : (sb + 1) * QB],
                        start=(ci == 0),
                        stop=(ci == NC - 1),
                    )
                nc.scalar.activation(
                    out=hT[:, ch, sb * QB : (sb + 1) * QB], in_=ps[:],
                    func=AF.Relu, bias=b1_sb[:, ch : ch + 1], scale=1.0,
                )
        for sc in range(NQB):
            for st4 in range(TPC):
                st = sc * TPC + st4
                psw = psum_w.tile([P, 1], F32, tag="small")
                for ch in range(NH):
                    nc.tensor.matmul(
                        psw[:],
                        lhsT=hT[:, ch, st * P : (st + 1) * P],
                        rhs=w2_sb[:, ch : ch + 1],
                        start=(ch == 0),
                        stop=(ch == NH - 1),
                    )
                wt = stage.tile([P, 1], F32, tag="wt", bufs=2)
                nc.scalar.activation(
                    out=wt[:], in_=psw[:], func=AF.Tanh, bias=b2h_sb[:], scale=0.5
                )
                nc.vector.tensor_scalar(
                    out=w_sb[:, st : st + 1], in0=wt[:], scalar1=0.5, scalar2=0.5,
                    op0=OP.mult, op1=OP.add,
                )
            # fused_v = w * v for this chunk
            vf = stage.tile([P, TPC, D], BF16, tag="v_nat", bufs=2)
            nc.sync.dma_start(
                out=vf[:],
                in_=v_d[sc * TPC : (sc + 1) * TPC].rearrange("t p d -> p t d"),
            )
            ov = stage.tile([P, TPC, D], BF16, tag="out_v", bufs=2)
            for st4 in range(TPC):
                st = sc * TPC + st4
                nc.vector.tensor_scalar_mul(
                    out=ov[:, st4, :], in0=vf[:, st4, :],
                    scalar1=w_sb[:, st : st + 1],
                )
            nc.scalar.dma_start(
                out=out[sc * TPC : (sc + 1) * TPC, :, 2 * D : 3 * D].rearrange(
                    "t p d -> p t d"
                ),
                in_=ov[:],
            )

    # ---- attention (fp8 DoubleRow), software-pipelined over q-blocks ----
    # One PSUM pool of 2-bank tiles serves both score pairs (Exp reads the
    # full [P, 2, 512]) and PV accumulators (o1 in bank 0, o2 in bank 1).
    with (
        tc.tile_pool(name="ppool", bufs=2) as ppool,
        tc.tile_pool(name="psum_pp", bufs=4, space="PSUM") as psum_pp,
    ):
        pts = {}

        def emit_scores(qb):
            pT = ppool.tile([P, NS, QB], F8, tag="pT")
            pts[qb] = pT
            for kp in range(NS // 2):
                ps = psum_pp.tile([P, 2, QB], F32, tag="pp")
                for i in range(2):
                    kt = 2 * kp + i
                    for j in range(2):
                        nc.tensor.matmul(
                            ps[:, i, :],
                            lhsT=kT[:, 2 * j : 2 * j + 2, kt * P : (kt + 1) * P],
                            rhs=qT[:, 2 * j : 2 * j + 2, qb * QB : (qb + 1) * QB],
                            start=(j == 0),
                            stop=(j == 1),
                            perf_mode=DR,
                        )
                nc.scalar.activation(
                    out=pT[:, 2 * kp : 2 * kp + 2, :], in_=ps[:], func=AF.Exp,
                    scale=SCALE, bias=expb_sb[:],
                )

        def emit_pv(qb):
            pT = pts.pop(qb)
            lf = stage.tile([P, TPC, D], BF16, tag="l_nat", bufs=2)
            nc.gpsimd.dma_start(
                out=lf[:],
                in_=l_d[qb * TPC : (qb + 1) * TPC].rearrange("t p d -> p t d"),
            )
            ol = stage.tile([P, TPC, D], BF16, tag="out_l", bufs=2)
            for qt in range(TPC):
                o12 = psum_pp.tile([P, 2, QB], F32, tag="pp")
                o1 = o12[:, 0, 0:N1]
                o2 = o12[:, 1, 0:N2]
                for t in range(NS // 2):
                    nc.tensor.matmul(
                        o1,
                        lhsT=pT[:, 2 * t : 2 * t + 2, qt * P : (qt + 1) * P],
                        rhs=v_sb[:, 2 * t : 2 * t + 2, 0:N1],
                        start=(t == 0),
                        stop=(t == NS // 2 - 1),
                        perf_mode=DR,
                    )
                for t in range(NS // 2):
                    nc.tensor.matmul(
                        o2,
                        lhsT=pT[:, 2 * t : 2 * t + 2, qt * P : (qt + 1) * P],
                        rhs=v_sb[:, 2 * t : 2 * t + 2, N1:DV],
                        start=(t == 0),
                        stop=(t == NS // 2 - 1),
                        perf_mode=DR,
                    )
                # lsc = (2-gl)*l + gl*bv (exact bv via the rowsum identity)
                lsc = stage.tile([P, D], F32, tag="lsc", bufs=2)
                nc.vector.scalar_tensor_tensor(
                    out=lsc[:], in0=lf[:, qt, :], scalar=2.0 - gl, in1=bv_bc[:],
                    op0=OP.mult, op1=OP.add,
                )
                rinv = stage.tile([P, 1], F32, tag="rinv", bufs=2)
                nc.vector.reciprocal(rinv[:], o2[:, N2 - 1 : N2])
                nc.vector.scalar_tensor_tensor(
                    out=ol[:, qt, 0:N1], in0=o1, scalar=rinv[:],
                    in1=lsc[:, 0:N1], op0=OP.mult, op1=OP.add,
                )
                nc.vector.scalar_tensor_tensor(
                    out=ol[:, qt, N1:D], in0=o2[:, 0 : N2 - 1], scalar=rinv[:],
                    in1=lsc[:, N1:D], op0=OP.mult, op1=OP.add,
                )
            nc.scalar.dma_start(
                out=out[qb * TPC : (qb + 1) * TPC, :, 0:D].rearrange(
                    "t p d -> p t d"
                ),
                in_=ol[:],
            )

        emit_scores(0)
        emit_scores(1)
        emit_pv(0)
        emit_scores(2)
        emit_pv(1)
        emit_scores(3)
        emit_pv(2)
        emit_pv(3)

    ctx.close()


def _execute(inputs, trace=False, **run_kwargs):
    f8 = ml_dtypes.float8_e4m3
    bf = ml_dtypes.bfloat16
    a = np.ascontiguousarray(np.asarray(inputs["a"], dtype=np.float32))
    v = np.ascontiguousarray(np.asarray(inputs["v"], dtype=np.float32))
    l = np.ascontiguousarray(np.asarray(inputs["l"], dtype=np.float32))
    Wq = np.asarray(inputs["Wq"], dtype=np.float32)
    Wk = np.asarray(inputs["Wk"], dtype=np.float32)
    Wv = np.asarray(inputs["Wv"], dtype=np.float32)
    W1 = np.asarray(inputs["W1"], dtype=np.float32)
    W2 = np.asarray(inputs["W2"], dtype=np.float32)
    bq = np.asarray(inputs["bq"], dtype=np.float32)
    bk = np.asarray(inputs["bk"], dtype=np.float32)
    bv = np.asarray(inputs["bv"], dtype=np.float32)
    b1 = np.asarray(inputs["b1"], dtype=np.float32)
    b2 = np.asarray(inputs["b2"], dtype=np.float32)
    alpha_a = float(np.asarray(inputs["alpha_a"]))
    alpha_l = float(np.asarray(inputs["alpha_l"]))

    gl = float(1.0 / (1.0 + math.exp(-alpha_l)))
    ga = float(1.0 / (1.0 + math.exp(-alpha_a)))
    b2val = float(b2.reshape(-1)[0])

    nc = build_kernel(gl, ga, b2val)

    shared = {
        "wq_d": np.ascontiguousarray(Wq.reshape(NC, P, D)).astype(f8),
        "wk_d": np.ascontiguousarray(Wk.reshape(NC, P, D)).astype(f8),
        "wv_d": np.ascontiguousarray(Wv.reshape(NC, P, D)).astype(f8),
        "w1_d": np.ascontiguousarray(W1.reshape(NC, P, HID)).astype(bf),
        "w2_d": np.ascontiguousarray(W2.reshape(NH, P).T).astype(bf),
        "bq_d": np.ascontiguousarray(bq.reshape(NC, P).T),
        "bk_d": np.ascontiguousarray(bk.reshape(NC, P).T),
        "b1_d": np.ascontiguousarray(b1.reshape(NH, P).T),
        "bvgl_d": np.ascontiguousarray((gl * bv).reshape(1, D)),
    }
    in_maps = []
    for i in range(B):
        m = dict(shared)
        m["aT_d"] = np.ascontiguousarray(a[i].T).astype(f8).reshape(NC, P, S)
        m["lT_d"] = np.ascontiguousarray(l[i].T).astype(f8).reshape(NC, P, S)
        m["vT_d"] = np.ascontiguousarray(v[i].T).astype(bf).reshape(NC, P, S)
        m["a_d"] = a[i].reshape(NS, P, D).astype(bf)
        m["l_d"] = l[i].reshape(NS, P, D).astype(bf)
        m["v_d"] = v[i].reshape(NS, P, D).astype(bf)
        in_maps.append(m)

    res = run_bass_kernel_spmd(
        nc, in_maps, core_ids=list(range(B)), trace=trace, **run_kwargs
    )
    outs = [
        res.results[i]["out"].astype(np.float32).reshape(S, 3 * D) for i in range(B)
    ]
    return np.stack(outs, axis=0), res


def kernel(**inputs) -> np.ndarray:
    out, _ = _execute(inputs, trace=False)
    return out


if __name__ == "__main__":
    print("kernel module OK")
